# revision 7
# baseline (speedup 1.0000x reference)
"""Multi-head causal attention (B=2, T=2048, C=4096, H=32) on 8 Trainium2
NeuronCores, tensor-parallel over heads (Megatron-style).

Per core m (4 heads each):
  phase 1: q/k/v projections from full x (weights column-sharded,
           host-pre-transposed into lhsT/rhs layouts). k-outer MM ordering so
           the first w-block streams against chunked weight/x DMAs (fast
           start). RoPE applied at PSUM eviction (all rotary freqs == 1.0, so
           cos/sin are per-position scalars; head_dim host-permuted to
           [evens, odds]; the half-swap runs through SBUF->SBUF DMA).
           v is evicted directly into a persistent SBUF tile (no DRAM trip).
  phase 2: attention per (head, batch) with scores TRANSPOSED [k, q]:
           u = exp(scale * sT) (no max subtraction needed at these scales).
           Blocks above the causal diagonal are skipped; diagonal blocks are
           column-trimmed so only the allowed query range streams through the
           PE. Softmax denominator accumulates on the vector engine in f32
           (one ones-matmul per query tile instead of per key block); the
           normalize + store pipeline is deferred one query tile so the PE
           never stalls on it. Per-head AllToAll redistributes o.T as soon as
           both batches of that head finish, overlapping the collective with
           attention of later heads.
  phase 3: y_rows = a_rows @ wo.T with the full wo (prefetched during
           phase 2); the ko accumulation consumes head 3 last so the final
           collective hides behind the first 24 matmuls of each output tile.
Host gathers the 8 row-slices; host does layout prep and the final concat.
"""

import os
import sys

import numpy as np

for _p in ("/opt/trn_rl_repo", "/root/.axon_site/_ro/trn_rl_repo"):
    if os.path.isdir(_p) and _p not in sys.path:
        sys.path.insert(0, _p)

import ml_dtypes

import concourse.bacc as bacc
import concourse.bass as bass
import concourse.mybir as mybir
import concourse.tile as tile
from concourse.bass_utils import run_bass_kernel_spmd

BF16 = ml_dtypes.bfloat16
P = 128
NCORES = 8
DT = mybir.dt.bfloat16
F32 = mybir.dt.float32
ActFn = mybir.ActivationFunctionType

FULL = dict(B=2, T=2048, C=4096, H=32, W=256, QT=512)


def _dims(cfg):
    B, T, C, H = cfg["B"], cfg["T"], cfg["C"], cfg["H"]
    W, QT = cfg["W"], cfg["QT"]
    HD = C // H
    assert HD == P
    HL = H // NCORES
    R = B * T
    RS = R // NCORES
    KO = C // P
    assert R % W == 0 and T % QT == 0 and QT % P == 0 and W % P == 0
    assert T % W == 0  # w-blocks may not straddle batches (cos/sin slicing)
    return B, T, C, H, HD, HL, R, RS, KO, W, QT


def build_nc(cfg=FULL):
    B, T, C, H, HD, HL, R, RS, KO, W, QT = _dims(cfg)
    NW = R // W
    NKT = T // P
    NQT = T // QT
    QB = QT // P  # key blocks per query tile on the diagonal
    SCALE = float(HD) ** -0.5

    nc = bacc.Bacc(None, num_devices=NCORES)

    xT = nc.dram_tensor("xT", [P, KO, R], DT, kind="ExternalInput")
    wqT = nc.dram_tensor("wqT", [P, KO, HL * HD], DT, kind="ExternalInput")
    wkT = nc.dram_tensor("wkT", [P, KO, HL * HD], DT, kind="ExternalInput")
    wvT = nc.dram_tensor("wvT", [P, KO, HL * HD], DT, kind="ExternalInput")
    woT = nc.dram_tensor("woT", [P, KO, C], DT, kind="ExternalInput")
    cosT = nc.dram_tensor("cosT", [P, T], DT, kind="ExternalInput")
    sinT = nc.dram_tensor("sinT", [P, T], DT, kind="ExternalInput")
    maskb = nc.dram_tensor("maskb", [P, P], DT, kind="ExternalInput")
    y = nc.dram_tensor("y", [RS, C], F32, kind="ExternalOutput")

    qT_d = nc.dram_tensor("qT_d", [P, HL, R], DT)
    kT_d = nc.dram_tensor("kT_d", [P, HL, R], DT)
    a2a_i = [nc.dram_tensor(f"a2a_i{h}", [NCORES, HD, RS], DT) for h in range(HL)]
    a2a_o = [nc.dram_tensor(f"a2a_o{h}", [NCORES, HD, RS], DT) for h in range(HL)]

    with tile.TileContext(nc) as tc:
        with (
            tc.tile_pool(name="tab", bufs=1) as tab,
            tc.tile_pool(name="vres", bufs=1) as vres,
        ):
            ones_sb = tab.tile([P, P], DT, tag="ones")
            nc.vector.memset(ones_sb[:], 1.0)
            mask_sb = tab.tile([P, P], DT, tag="mask")
            nc.gpsimd.dma_start(mask_sb[:], maskb[:])
            cos_sb = tab.tile([P, T], DT, tag="cos")
            sin_sb = tab.tile([P, T], DT, tag="sin")
            nc.gpsimd.dma_start(cos_sb[:], cosT[:])
            nc.gpsimd.dma_start(sin_sb[:], sinT[:])
            v_sb = vres.tile([P, R // P, HL * HD], DT, tag="v")

            # ------------- phase 1: q/k/v projections + rope -------------
            with (
                tc.tile_pool(name="wp", bufs=1) as wp,
                tc.tile_pool(name="xp", bufs=2) as xp,
                tc.tile_pool(name="ev1", bufs=3) as ev1,
                tc.tile_pool(name="ps1", bufs=1, space="PSUM") as ps1,
                tc.tile_pool(name="psv", bufs=2, space="PSUM") as psv,
            ):
                wq_sb = wp.tile([P, KO, HL * HD], DT, tag="wq")
                wk_sb = wp.tile([P, KO, HL * HD], DT, tag="wk")
                wv_sb = wp.tile([P, KO, HL * HD], DT, tag="wv")
                xw0 = xp.tile([P, KO, W], DT, tag="xw")
                # chunked startup loads on parallel queues: PE starts after
                # the first chunks land instead of after whole-tile loads.
                for k in range(KO):
                    nc.sync.dma_start(wq_sb[:, k], wqT[:, k])
                for k in range(KO):
                    nc.scalar.dma_start(xw0[:, k], xT[:, k, 0:W])
                for k in range(KO):
                    nc.scalar.dma_start(wk_sb[:, k], wkT[:, k])
                for k in range(KO):
                    nc.sync.dma_start(wv_sb[:, k], wvT[:, k])

                for w in range(NW):
                    if w == 0:
                        xw = xw0
                    else:
                        xw = xp.tile([P, KO, W], DT, tag="xw")
                        nc.sync.dma_start(xw[:], xT[:, :, w * W:(w + 1) * W])
                    rsl = slice(w * W, (w + 1) * W)
                    t0 = (w * W) % T
                    tsl = slice(t0, t0 + W)

                    for wsb, dst in ((wq_sb, qT_d), (wk_sb, kT_d)):
                        pts = [ps1.tile([P, W], F32, tag=f"p{h}",
                                        name=f"pqk{h}")
                               for h in range(HL)]
                        for k in range(KO):
                            for h in range(HL):
                                nc.tensor.matmul(
                                    pts[h][:],
                                    wsb[:, k, h * HD:(h + 1) * HD], xw[:, k],
                                    start=(k == 0), stop=(k == KO - 1),
                                )
                        for h in range(HL):
                            # rope: rot = raw*cos + swap(raw)*sin (sign-split
                            # sin); engines need same-start-partition
                            # operands, so the half-swap goes through
                            # SBUF->SBUF DMA.
                            raw = ev1.tile([P, W], DT, tag="raw")
                            nc.scalar.activation(raw[:], pts[h][:], ActFn.Copy)
                            sw = ev1.tile([P, W], DT, tag="sw")
                            nc.sync.dma_start(sw[0:64, :], raw[64:128, :])
                            nc.sync.dma_start(sw[64:128, :], raw[0:64, :])
                            t1 = ev1.tile([P, W], DT, tag="t1")
                            nc.vector.tensor_tensor(
                                t1[:], sw[:], sin_sb[:, tsl],
                                mybir.AluOpType.mult)
                            rot = ev1.tile([P, W], DT, tag="rot")
                            nc.vector.tensor_tensor(
                                rot[:], raw[:], cos_sb[:, tsl],
                                mybir.AluOpType.mult)
                            nc.vector.tensor_tensor(
                                rot[:], rot[:], t1[:], mybir.AluOpType.add)
                            nc.scalar.dma_start(dst[:, h, rsl], rot[:])

                    for rs_ in range(W // P):
                        pt = psv.tile([P, HL * HD], F32, tag="pv")
                        for k in range(KO):
                            nc.tensor.matmul(
                                pt[:], xw[:, k, rs_ * P:(rs_ + 1) * P],
                                wv_sb[:, k],
                                start=(k == 0), stop=(k == KO - 1),
                            )
                        nc.scalar.activation(
                            v_sb[:, w * (W // P) + rs_, :], pt[:], ActFn.Copy)

            # ------------- phases 2+3 -------------
            with (
                tc.tile_pool(name="a3", bufs=1) as a3,
                tc.tile_pool(name="wop", bufs=2) as wop,
            ):
                aT_sb = a3.tile([P, HL, NCORES, RS], DT, tag="aT")
                # prefetch the first two wo column-blocks during attention
                NCB = C // QT
                wots = []
                for cb in range(2):
                    wot = wop.tile([P, KO, QT], DT, tag="wot")
                    nc.gpsimd.dma_start(
                        wot[:], woT[:, :, cb * QT:(cb + 1) * QT])
                    wots.append(wot)

                with (
                    tc.tile_pool(name="att", bufs=2) as att,
                    tc.tile_pool(name="up", bufs=3) as up,
                    tc.tile_pool(name="ps2", bufs=3, space="PSUM") as ps2,
                    tc.tile_pool(name="pso", bufs=3, space="PSUM") as pso,
                    tc.tile_pool(name="psd", bufs=2, space="PSUM") as psd,
                ):
                    pending = None

                    def flush_pending():
                        nonlocal pending
                        if pending is None:
                            return
                        usb_t, po_t, hh, slot = pending
                        pd = psd.tile([P, QT], F32, tag="pd")
                        nc.tensor.matmul(
                            pd[:], ones_sb[:], usb_t[:], start=True, stop=True)
                        rec = att.tile([P, QT], F32, tag="rec")
                        nc.vector.reciprocal(rec[:], pd[:])
                        ot = att.tile([P, QT], DT, tag="ot")
                        nc.vector.tensor_tensor(
                            ot[:], po_t[:], rec[:], mybir.AluOpType.mult)
                        nc.scalar.dma_start(a2a_i[hh][slot], ot[:])
                        pending = None

                    for h in range(HL):
                        for b in range(B):
                            kTb = att.tile([P, T], DT, tag="kTb")
                            nc.sync.dma_start(
                                kTb[:], kT_d[:, h, b * T:(b + 1) * T])
                            for qt in range(NQT):
                                qTt = att.tile([P, QT], DT, tag="qTt")
                                nc.sync.dma_start(
                                    qTt[:],
                                    qT_d[:, h,
                                         b * T + qt * QT:b * T + (qt + 1) * QT])
                                po = pso.tile([P, QT], F32, tag="po")
                                usum = att.tile([P, QT], F32, tag="usum")
                                nkt = (qt + 1) * QB
                                for kt in range(nkt):
                                    j = kt - qt * QB
                                    c0 = 128 * j if j > 0 else 0
                                    cs = slice(c0, QT)
                                    pS = ps2.tile([P, QT], F32, tag="pS")
                                    nc.tensor.matmul(
                                        pS[:, cs],
                                        kTb[:, kt * P:(kt + 1) * P],
                                        qTt[:, cs],
                                        start=True, stop=True,
                                    )
                                    u = up.tile([P, QT], DT, tag="u")
                                    nc.scalar.activation(
                                        u[:, cs], pS[:, cs], ActFn.Exp,
                                        scale=SCALE)
                                    if j >= 0:  # mask the 128-wide diagonal
                                        nc.vector.tensor_tensor(
                                            u[:, c0:c0 + P], u[:, c0:c0 + P],
                                            mask_sb[:], mybir.AluOpType.mult)
                                    nc.tensor.matmul(
                                        po[:, cs],
                                        v_sb[:, b * NKT + kt,
                                             h * HD:(h + 1) * HD],
                                        u[:, cs],
                                        start=(kt == 0), stop=(kt == nkt - 1))
                                    if kt == 0:
                                        nc.vector.tensor_copy(usum[:], u[:])
                                    else:
                                        nc.vector.tensor_tensor(
                                            usum[:, cs], usum[:, cs], u[:, cs],
                                            mybir.AluOpType.add)
                                # normalize of the PREVIOUS tile goes here so
                                # its ones-matmul runs behind this tile's MMs
                                flush_pending()
                                usb = att.tile([P, QT], DT, tag="usb")
                                nc.vector.tensor_copy(usb[:], usum[:])
                                pending = (usb, po, h, b * NQT + qt)
                        flush_pending()
                        nc.gpsimd.collective_compute(
                            "AllToAll",
                            mybir.AluOpType.bypass,
                            replica_groups=[list(range(NCORES))],
                            ins=[a2a_i[h][:]],
                            outs=[a2a_o[h][:]],
                        )
                        nc.gpsimd.dma_start(
                            aT_sb[:, h, :, :],
                            a2a_o[h][:].rearrange("s d r -> d s r"))

                # ------------- phase 3: output projection -------------
                with (
                    tc.tile_pool(name="yp", bufs=3) as yp,
                    tc.tile_pool(name="ps3", bufs=2, space="PSUM") as ps3,
                ):
                    for cb in range(NCB):
                        if cb < 2:
                            wot = wots[cb]
                        else:
                            wot = wop.tile([P, KO, QT], DT, tag="wot")
                            nc.gpsimd.dma_start(
                                wot[:], woT[:, :, cb * QT:(cb + 1) * QT])
                        for rs_ in range(RS // P):
                            pt = ps3.tile([P, QT], F32, tag="py")
                            n = 0
                            for i in range(HL):  # head 3 last: its a2a hides
                                for s in range(NCORES):
                                    ko = s * HL + i
                                    nc.tensor.matmul(
                                        pt[:],
                                        aT_sb[:, i, s,
                                              rs_ * P:(rs_ + 1) * P],
                                        wot[:, ko],
                                        start=(n == 0),
                                        stop=(n == HL * NCORES - 1),
                                    )
                                    n += 1
                            yt = yp.tile([P, QT], F32, tag="yt")
                            nc.scalar.activation(yt[:], pt[:], ActFn.Copy)
                            nc.scalar.dma_start(
                                y[rs_ * P:(rs_ + 1) * P,
                                  cb * QT:(cb + 1) * QT], yt[:])

    nc.compile()
    return nc


def _as_lhsT_tiles(w):
    """[M, K] row-major -> [P, K//P, M]: out[p, ko, m] = w[m, ko*P + p]."""
    M, K = w.shape
    return np.ascontiguousarray(
        w.reshape(M, K // P, P).transpose(2, 1, 0)).astype(BF16)


def prep_inputs(x, wq, wk, wv, wo, cfg=FULL):
    B, T, C, H, HD, HL, R, RS, KO, W, QT = _dims(cfg)
    rope_perm = np.concatenate([np.arange(0, HD, 2), np.arange(1, HD, 2)])

    xflat = np.ascontiguousarray(x.reshape(R, C))
    xT = _as_lhsT_tiles(xflat)                       # [P, KO, R]
    woT = _as_lhsT_tiles(wo)                         # [P, KO, C]

    t = np.arange(T, dtype=np.float64)
    cosT = np.broadcast_to(np.cos(t), (P, T)).astype(BF16)
    sin_row = np.sin(t)
    sinT = np.empty((P, T), np.float64)
    sinT[0:64, :] = -sin_row
    sinT[64:128, :] = sin_row
    sinT = sinT.astype(BF16)

    # triangular causal mask for the 128-wide diagonal: allowed iff c >= p
    cc = np.arange(P)
    maskb = (cc[None, :] >= cc[:, None]).astype(BF16)

    per_core = []
    for m in range(NCORES):
        sl = slice(m * HL * HD, (m + 1) * HL * HD)
        wq_m = wq[sl].reshape(HL, HD, C)[:, rope_perm, :].reshape(HL * HD, C)
        wk_m = wk[sl].reshape(HL, HD, C)[:, rope_perm, :].reshape(HL * HD, C)
        per_core.append(dict(
            xT=xT,
            wqT=_as_lhsT_tiles(wq_m),
            wkT=_as_lhsT_tiles(wk_m),
            wvT=_as_lhsT_tiles(wv[sl]),
            woT=woT,
            cosT=cosT,
            sinT=sinT,
            maskb=maskb,
        ))
    return per_core


_NC_CACHE = None
LAST_EXEC_NS = None
LAST_RESULT = None


def kernel(x, wq, wk, wv, wo):
    global _NC_CACHE, LAST_EXEC_NS, LAST_RESULT
    cfg = FULL
    B, T, C = cfg["B"], cfg["T"], cfg["C"]
    if _NC_CACHE is None:
        _NC_CACHE = build_nc(cfg)
    nc = _NC_CACHE
    in_maps = prep_inputs(
        np.asarray(x, np.float32), np.asarray(wq, np.float32),
        np.asarray(wk, np.float32), np.asarray(wv, np.float32),
        np.asarray(wo, np.float32), cfg)
    res = run_bass_kernel_spmd(nc, in_maps, core_ids=list(range(NCORES)))
    LAST_RESULT = res
    if res.exec_time_ns is not None:
        LAST_EXEC_NS = res.exec_time_ns
    y = np.concatenate([r["y"] for r in res.results], axis=0)
    return y.reshape(B, T, C).astype(np.float32)


# revision 11
# speedup vs baseline: 1.0446x; 1.0446x over previous
"""Multi-head causal attention (B=2, T=2048, C=4096, H=32) on 8 Trainium2
NeuronCores, tensor-parallel over heads (Megatron-style).

Per core m (4 heads each):
  phase 1: q/k/v projections from full x (weights column-sharded,
           host-pre-transposed into lhsT/rhs layouts). k-outer MM ordering so
           the first w-block streams against chunked weight/x DMAs (fast
           start). RoPE applied at PSUM eviction (all rotary freqs == 1.0, so
           cos/sin are per-position scalars; head_dim host-permuted to
           [evens, odds]; the half-swap runs through SBUF->SBUF DMA).
           v is evicted directly into a persistent SBUF tile (no DRAM trip).
  phase 2: attention per (head, batch) with scores TRANSPOSED [k, q]:
           u = exp(scale * sT) (no max subtraction needed at these scales).
           Blocks above the causal diagonal are skipped; diagonal blocks are
           column-trimmed. The causal mask is applied ON the tensor engine:
           a constant [-1e30 upper-triangle] stationary matrix against an
           identity accumulates into the diagonal 128-block of the scores
           PSUM (reference's additive NEG_INF semantics) so no vector op
           sits between exp and the PV matmul. Softmax denominator
           accumulates in PSUM via trimmed all-ones matmuls.
  a2a:     TWO AllToAlls (heads {0,1} after their batches finish mid-phase-2,
           heads {2,3} at the end) - per-collective cost here is ~fixed, so
           the first hides under attention of heads 2/3 and the second under
           the first half of phase 3.
  phase 3: y_rows = a_rows @ wo.T in two half-contraction passes: pass A
           accumulates heads {0,1} into f32 SBUF partials while a2a(23) is
           in flight; pass B adds heads {2,3} and stores. wo streams twice
           (64 MiB, amortized under compute).
Host gathers the 8 row-slices; host does layout prep and the final concat.
"""

import os
import sys

import numpy as np

for _p in ("/opt/trn_rl_repo", "/root/.axon_site/_ro/trn_rl_repo"):
    if os.path.isdir(_p) and _p not in sys.path:
        sys.path.insert(0, _p)

import ml_dtypes

import concourse.bacc as bacc
import concourse.bass as bass
import concourse.mybir as mybir
import concourse.tile as tile
from concourse.bass_utils import run_bass_kernel_spmd

BF16 = ml_dtypes.bfloat16
P = 128
NCORES = 8
DT = mybir.dt.bfloat16
F32 = mybir.dt.float32
ActFn = mybir.ActivationFunctionType
NEG = -1e30

FULL = dict(B=2, T=2048, C=4096, H=32, W=256, QT=512)


def _dims(cfg):
    B, T, C, H = cfg["B"], cfg["T"], cfg["C"], cfg["H"]
    W, QT = cfg["W"], cfg["QT"]
    HD = C // H
    assert HD == P
    HL = H // NCORES
    R = B * T
    RS = R // NCORES
    KO = C // P
    assert R % W == 0 and T % QT == 0 and QT % P == 0 and W % P == 0
    assert T % W == 0  # w-blocks may not straddle batches (cos/sin slicing)
    return B, T, C, H, HD, HL, R, RS, KO, W, QT


def build_nc(cfg=FULL):
    B, T, C, H, HD, HL, R, RS, KO, W, QT = _dims(cfg)
    NW = R // W
    NWB = NW // B  # w-blocks per batch
    NKT = T // P
    NQT = T // QT
    QB = QT // P
    NCB = C // QT
    SCALE = float(HD) ** -0.5

    nc = bacc.Bacc(None, num_devices=NCORES)

    xT = nc.dram_tensor("xT", [P, KO, R], DT, kind="ExternalInput")
    wqT = nc.dram_tensor("wqT", [P, KO, HL * HD], DT, kind="ExternalInput")
    wkT = nc.dram_tensor("wkT", [P, KO, HL * HD], DT, kind="ExternalInput")
    wvT = nc.dram_tensor("wvT", [P, KO, HL * HD], DT, kind="ExternalInput")
    woT = nc.dram_tensor("woT", [P, KO, C], DT, kind="ExternalInput")
    cosT = nc.dram_tensor("cosT", [P, T], DT, kind="ExternalInput")
    sinT = nc.dram_tensor("sinT", [P, T], DT, kind="ExternalInput")
    negT = nc.dram_tensor("negT", [P, P], DT, kind="ExternalInput")
    idT = nc.dram_tensor("idT", [P, P], DT, kind="ExternalInput")
    y = nc.dram_tensor("y", [RS, C], F32, kind="ExternalOutput")

    qT_d = nc.dram_tensor("qT_d", [P, HL, R], DT)
    kT_d = nc.dram_tensor("kT_d", [P, HL, R], DT)
    # two a2a groups: heads {0,1} and heads {2,3}
    a2a_i = [nc.dram_tensor(f"a2a_i{g}", [NCORES, 2 * HD, RS], DT)
             for g in range(2)]
    a2a_o = [nc.dram_tensor(f"a2a_o{g}", [NCORES, 2 * HD, RS], DT)
             for g in range(2)]

    with tile.TileContext(nc) as tc:
        with (
            tc.tile_pool(name="tab", bufs=1) as tab,
            tc.tile_pool(name="attpre", bufs=1) as attpre,
            tc.tile_pool(name="vres", bufs=1) as vres,
        ):
            ones_sb = tab.tile([P, P], DT, tag="ones")
            nc.vector.memset(ones_sb[:], 1.0)
            neg_sb = tab.tile([P, P], DT, tag="neg")
            nc.gpsimd.dma_start(neg_sb[:], negT[:])
            id_sb = tab.tile([P, P], DT, tag="id")
            nc.gpsimd.dma_start(id_sb[:], idT[:])
            # first-attention tiles in a pool that can't overlap the phase-1
            # pools: their loads run mid-phase-1, killing the transition gap
            kTb0 = attpre.tile([P, T], DT, tag="kTb0")
            qTt0 = attpre.tile([P, QT], DT, tag="qTt0")
            v_sb = vres.tile([P, R // P, HL * HD], DT, tag="v")

            if True:
                # ------------- phase 1: q/k/v projections + rope -------------
                with (
                    tc.tile_pool(name="cs1", bufs=1) as cs1,
                    tc.tile_pool(name="wp", bufs=1) as wp,
                    tc.tile_pool(name="xp", bufs=2) as xp,
                    tc.tile_pool(name="ev1", bufs=3) as ev1,
                    tc.tile_pool(name="ps1", bufs=1, space="PSUM") as ps1,
                    tc.tile_pool(name="psv", bufs=2, space="PSUM") as psv,
                ):
                    cos_sb = cs1.tile([P, T], DT, tag="cos")
                    sin_sb = cs1.tile([P, T], DT, tag="sin")
                    nc.gpsimd.dma_start(cos_sb[:], cosT[:])
                    nc.gpsimd.dma_start(sin_sb[:], sinT[:])
                    wq_sb = wp.tile([P, KO, HL * HD], DT, tag="wq")
                    wk_sb = wp.tile([P, KO, HL * HD], DT, tag="wk")
                    wv_sb = wp.tile([P, KO, HL * HD], DT, tag="wv")
                    xw0 = xp.tile([P, KO, W], DT, tag="xw")
                    # chunked startup loads on parallel queues
                    for k in range(KO):
                        nc.sync.dma_start(wq_sb[:, k], wqT[:, k])
                    for k in range(KO):
                        nc.scalar.dma_start(xw0[:, k], xT[:, k, 0:W])
                    for k in range(KO):
                        nc.scalar.dma_start(wk_sb[:, k], wkT[:, k])
                    for k in range(KO):
                        nc.sync.dma_start(wv_sb[:, k], wvT[:, k])

                    for w in range(NW):
                        if w == 0:
                            xw = xw0
                        else:
                            xw = xp.tile([P, KO, W], DT, tag="xw")
                            nc.sync.dma_start(
                                xw[:], xT[:, :, w * W:(w + 1) * W])
                        rsl = slice(w * W, (w + 1) * W)
                        t0 = (w * W) % T
                        tsl = slice(t0, t0 + W)

                        for wsb, dst in ((wq_sb, qT_d), (wk_sb, kT_d)):
                            pts = [ps1.tile([P, W], F32, tag=f"p{h}",
                                            name=f"pqk{h}")
                                   for h in range(HL)]
                            for k in range(KO):
                                for h in range(HL):
                                    nc.tensor.matmul(
                                        pts[h][:],
                                        wsb[:, k, h * HD:(h + 1) * HD],
                                        xw[:, k],
                                        start=(k == 0), stop=(k == KO - 1),
                                    )
                            for h in range(HL):
                                raw = ev1.tile([P, W], DT, tag="raw")
                                nc.scalar.activation(
                                    raw[:], pts[h][:], ActFn.Copy)
                                sw = ev1.tile([P, W], DT, tag="sw")
                                nc.sync.dma_start(sw[0:64, :], raw[64:128, :])
                                nc.sync.dma_start(sw[64:128, :], raw[0:64, :])
                                t1 = ev1.tile([P, W], DT, tag="t1")
                                nc.vector.tensor_tensor(
                                    t1[:], sw[:], sin_sb[:, tsl],
                                    mybir.AluOpType.mult)
                                rot = ev1.tile([P, W], DT, tag="rot")
                                nc.vector.tensor_tensor(
                                    rot[:], raw[:], cos_sb[:, tsl],
                                    mybir.AluOpType.mult)
                                nc.vector.tensor_tensor(
                                    rot[:], rot[:], t1[:],
                                    mybir.AluOpType.add)
                                nc.scalar.dma_start(dst[:, h, rsl], rot[:])

                        for rs_ in range(W // P):
                            pt = psv.tile([P, HL * HD], F32, tag="pv")
                            for k in range(KO):
                                nc.tensor.matmul(
                                    pt[:], xw[:, k, rs_ * P:(rs_ + 1) * P],
                                    wv_sb[:, k],
                                    start=(k == 0), stop=(k == KO - 1),
                                )
                            nc.scalar.activation(
                                v_sb[:, w * (W // P) + rs_, :], pt[:],
                                ActFn.Copy)

                        if w == NWB - 1:
                            # batch-0 projections done: prefetch the first
                            # attention operands now
                            nc.sync.dma_start(kTb0[:], kT_d[:, 0, 0:T])
                            nc.sync.dma_start(qTt0[:], qT_d[:, 0, 0:QT])

            with (
                tc.tile_pool(name="a3", bufs=1) as a3,
                tc.tile_pool(name="wop", bufs=2) as wop,
            ):
                # aT layout: [dim, core, local-head-in-group, row]; the two
                # groups land from their own collectives
                aT01 = a3.tile([P, NCORES, 2, RS], DT, tag="aT01")
                aT23 = a3.tile([P, NCORES, 2, RS], DT, tag="aT23")
                # prefetch the first two wo blocks (gpsimd is idle until the
                # first collective; space reuses released phase-1 pools)
                wots = []
                for cb in range(2):
                    wot = wop.tile([P, KO, QT], DT, tag="wot", name="wotp")
                    nc.gpsimd.dma_start(
                        wot[:], woT[:, :, cb * QT:(cb + 1) * QT])
                    wots.append(wot)

                # ------------- phase 2: attention -------------
                with (
                    tc.tile_pool(name="att", bufs=2) as att,
                    tc.tile_pool(name="qp", bufs=3) as qp,
                    tc.tile_pool(name="up", bufs=3) as up,
                    tc.tile_pool(name="ps2", bufs=3, space="PSUM") as ps2,
                    tc.tile_pool(name="pso", bufs=2, space="PSUM") as pso,
                    tc.tile_pool(name="psd", bufs=2, space="PSUM") as psd,
                ):
                    for h in range(HL):
                        for b in range(B):
                            if h == 0 and b == 0:
                                kTb = kTb0
                            else:
                                kTb = att.tile([P, T], DT, tag="kTb")
                                nc.sync.dma_start(
                                    kTb[:], kT_d[:, h, b * T:(b + 1) * T])
                            for qt in range(NQT):
                                if h == 0 and b == 0 and qt == 0:
                                    qTt = qTt0
                                else:
                                    qTt = qp.tile([P, QT], DT, tag="qTt")
                                    nc.sync.dma_start(
                                        qTt[:],
                                        qT_d[:, h, b * T + qt * QT:
                                             b * T + (qt + 1) * QT])
                                po = pso.tile([P, QT], F32, tag="po")
                                pd = psd.tile([P, QT], F32, tag="pd")
                                nkt = (qt + 1) * QB
                                for kt in range(nkt):
                                    j = kt - qt * QB
                                    c0 = P * j if j > 0 else 0
                                    cs = slice(c0, QT)
                                    diag = j >= 0
                                    pS = ps2.tile([P, QT], F32, tag="pS")
                                    nc.tensor.matmul(
                                        pS[:, cs],
                                        kTb[:, kt * P:(kt + 1) * P],
                                        qTt[:, cs],
                                        start=True, stop=not diag,
                                    )
                                    if diag:  # -1e30 upper triangle, on PE
                                        nc.tensor.matmul(
                                            pS[:, c0:c0 + P], neg_sb[:],
                                            id_sb[:],
                                            start=False, stop=True,
                                        )
                                    u = up.tile([P, QT], DT, tag="u")
                                    nc.scalar.activation(
                                        u[:, cs], pS[:, cs], ActFn.Exp,
                                        scale=SCALE)
                                    first, last = (kt == 0), (kt == nkt - 1)
                                    nc.tensor.matmul(
                                        po[:, cs],
                                        v_sb[:, b * NKT + kt,
                                             h * HD:(h + 1) * HD],
                                        u[:, cs], start=first, stop=last)
                                    nc.tensor.matmul(
                                        pd[:, cs], ones_sb[:], u[:, cs],
                                        start=first, stop=last)
                                rec = att.tile([P, QT], F32, tag="rec")
                                nc.vector.reciprocal(rec[:], pd[:])
                                ot = att.tile([P, QT], DT, tag="ot")
                                nc.vector.tensor_tensor(
                                    ot[:], po[:], rec[:],
                                    mybir.AluOpType.mult)
                                g, hh = divmod(h, 2)
                                nc.scalar.dma_start(
                                    a2a_i[g][b * NQT + qt,
                                             hh * HD:(hh + 1) * HD, :],
                                    ot[:])
                        if h == 1 or h == 3:
                            g = h // 2
                            nc.gpsimd.collective_compute(
                                "AllToAll",
                                mybir.AluOpType.bypass,
                                replica_groups=[list(range(NCORES))],
                                ins=[a2a_i[g][:]],
                                outs=[a2a_o[g][:]],
                            )
                            nc.gpsimd.dma_start(
                                (aT01 if g == 0 else aT23)[:],
                                a2a_o[g][:].rearrange("s (i d) r -> d s i r",
                                                      d=P))

                # ---------- phase 3: output projection, two passes ----------
                with (
                    tc.tile_pool(name="yacc", bufs=1) as yacc,
                    tc.tile_pool(name="yp", bufs=3) as yp,
                    tc.tile_pool(name="ps3", bufs=2, space="PSUM") as ps3,
                ):
                    ya_tiles = []
                    # pass A: heads {0,1} -> bf16 partials in SBUF (runs
                    # while a2a of heads {2,3} is in flight)
                    for cb in range(NCB):
                        if cb < 2:
                            wot = wots[cb]
                        else:
                            wot = wop.tile([P, KO, QT], DT, tag="wot",
                                           name="wota")
                            nc.sync.dma_start(
                                wot[:], woT[:, :, cb * QT:(cb + 1) * QT])
                        for rs_ in range(RS // P):
                            pt = ps3.tile([P, QT], F32, tag="py")
                            n = 0
                            for s in range(NCORES):
                                for i in range(2):
                                    nc.tensor.matmul(
                                        pt[:],
                                        aT01[:, s, i, rs_ * P:(rs_ + 1) * P],
                                        wot[:, s * HL + i],
                                        start=(n == 0),
                                        stop=(n == 2 * NCORES - 1),
                                    )
                                    n += 1
                            ya = yacc.tile([P, QT], DT, tag=f"ya{cb}_{rs_}",
                                           name=f"ya{cb}_{rs_}")
                            nc.scalar.activation(ya[:], pt[:], ActFn.Copy)
                            ya_tiles.append(ya)
                    # pass B: heads {2,3} + merge + store
                    for cb in range(NCB):
                        wot = wop.tile([P, KO, QT], DT, tag="wot",
                                       name="wotb")
                        nc.gpsimd.dma_start(
                            wot[:], woT[:, :, cb * QT:(cb + 1) * QT])
                        for rs_ in range(RS // P):
                            pt = ps3.tile([P, QT], F32, tag="py")
                            n = 0
                            for s in range(NCORES):
                                for i in range(2):
                                    nc.tensor.matmul(
                                        pt[:],
                                        aT23[:, s, i, rs_ * P:(rs_ + 1) * P],
                                        wot[:, s * HL + 2 + i],
                                        start=(n == 0),
                                        stop=(n == 2 * NCORES - 1),
                                    )
                                    n += 1
                            yt = yp.tile([P, QT], F32, tag="yt")
                            nc.vector.tensor_tensor(
                                yt[:], pt[:],
                                ya_tiles[cb * (RS // P) + rs_][:],
                                mybir.AluOpType.add)
                            nc.scalar.dma_start(
                                y[rs_ * P:(rs_ + 1) * P,
                                  cb * QT:(cb + 1) * QT], yt[:])

    nc.compile()
    return nc


def _as_lhsT_tiles(w):
    """[M, K] row-major -> [P, K//P, M]: out[p, ko, m] = w[m, ko*P + p]."""
    M, K = w.shape
    return np.ascontiguousarray(
        w.reshape(M, K // P, P).transpose(2, 1, 0)).astype(BF16)


def prep_inputs(x, wq, wk, wv, wo, cfg=FULL):
    B, T, C, H, HD, HL, R, RS, KO, W, QT = _dims(cfg)
    rope_perm = np.concatenate([np.arange(0, HD, 2), np.arange(1, HD, 2)])

    xflat = np.ascontiguousarray(x.reshape(R, C))
    xT = _as_lhsT_tiles(xflat)                       # [P, KO, R]
    woT = _as_lhsT_tiles(wo)                         # [P, KO, C]

    t = np.arange(T, dtype=np.float64)
    cosT = np.broadcast_to(np.cos(t), (P, T)).astype(BF16)
    sin_row = np.sin(t)
    sinT = np.empty((P, T), np.float64)
    sinT[0:64, :] = -sin_row
    sinT[64:128, :] = sin_row
    sinT = sinT.astype(BF16)

    # mask matmul constants: out[m, c] = negT[c, m] must be NEG iff c < m
    cc = np.arange(P)
    negT = np.where(cc[:, None] < cc[None, :], NEG, 0.0).astype(BF16)
    idT = np.eye(P).astype(BF16)

    per_core = []
    for m in range(NCORES):
        sl = slice(m * HL * HD, (m + 1) * HL * HD)
        wq_m = wq[sl].reshape(HL, HD, C)[:, rope_perm, :].reshape(HL * HD, C)
        wk_m = wk[sl].reshape(HL, HD, C)[:, rope_perm, :].reshape(HL * HD, C)
        per_core.append(dict(
            xT=xT,
            wqT=_as_lhsT_tiles(wq_m),
            wkT=_as_lhsT_tiles(wk_m),
            wvT=_as_lhsT_tiles(wv[sl]),
            woT=woT,
            cosT=cosT,
            sinT=sinT,
            negT=negT,
            idT=idT,
        ))
    return per_core


_NC_CACHE = None
LAST_EXEC_NS = None
LAST_RESULT = None


def kernel(x, wq, wk, wv, wo):
    global _NC_CACHE, LAST_EXEC_NS, LAST_RESULT
    cfg = FULL
    B, T, C = cfg["B"], cfg["T"], cfg["C"]
    if _NC_CACHE is None:
        _NC_CACHE = build_nc(cfg)
    nc = _NC_CACHE
    in_maps = prep_inputs(
        np.asarray(x, np.float32), np.asarray(wq, np.float32),
        np.asarray(wk, np.float32), np.asarray(wv, np.float32),
        np.asarray(wo, np.float32), cfg)
    res = run_bass_kernel_spmd(nc, in_maps, core_ids=list(range(NCORES)))
    LAST_RESULT = res
    if res.exec_time_ns is not None:
        LAST_EXEC_NS = res.exec_time_ns
    y = np.concatenate([r["y"] for r in res.results], axis=0)
    return y.reshape(B, T, C).astype(np.float32)


# revision 12
# speedup vs baseline: 1.0835x; 1.0372x over previous
"""Multi-head causal attention (B=2, T=2048, C=4096, H=32) on 8 Trainium2
NeuronCores, tensor-parallel over heads (Megatron-style).

Per core m (4 heads each):
  phase 1: q/k/v projections from full x (weights column-sharded,
           host-pre-transposed into lhsT/rhs layouts). k-outer MM ordering
           streams against chunked (4-ko) weight/x DMAs on parallel queues so
           the PE starts ~1us in. RoPE applied at PSUM eviction (all rotary
           freqs == 1.0, so cos/sin are per-position scalars; head_dim
           host-permuted to [evens, odds]; the half-swap runs through
           SBUF->SBUF DMA). v is evicted directly into a persistent SBUF
           tile (no DRAM trip).
  phase 2: attention per (head, batch) with scores TRANSPOSED [k, q]:
           u = exp(scale * sT) (no max subtraction needed at these scales).
           Blocks above the causal diagonal are skipped; diagonal blocks are
           column-trimmed. The causal mask is applied ON the tensor engine: a
           constant [-1e30 upper-triangle] stationary matrix against identity
           accumulates into the diagonal 128-block of the scores PSUM
           (reference's additive NEG_INF semantics). The PV/denominator
           matmuls are emitted one block behind the scores matmul so the exp
           latency hides behind the next score block. Softmax denominator
           accumulates in PSUM via trimmed all-ones matmuls; normalization
           uses the single-op approx reciprocal.
  a2a:     TWO AllToAlls (heads {0,1} fire mid-phase-2, heads {2,3} at the
           end) - per-collective cost here is ~fixed, so the first hides
           under attention of heads 2/3 and the second under the first half
           of phase 3.
  phase 3: y_rows = a_rows @ wo.T in two half-contraction passes: pass A
           accumulates heads {0,1} into bf16 SBUF partials while a2a(23) is
           in flight; pass B adds heads {2,3} and stores. wo streams twice
           as 16 half-tiles per pass with 4 buffers so loads stay ahead of
           consumption.
Host gathers the 8 row-slices; host does layout prep and the final concat.
"""

import os
import sys

import numpy as np

for _p in ("/opt/trn_rl_repo", "/root/.axon_site/_ro/trn_rl_repo"):
    if os.path.isdir(_p) and _p not in sys.path:
        sys.path.insert(0, _p)

import ml_dtypes

import concourse.bacc as bacc
import concourse.bass as bass
import concourse.mybir as mybir
import concourse.tile as tile
from concourse.bass_utils import run_bass_kernel_spmd

BF16 = ml_dtypes.bfloat16
P = 128
NCORES = 8
DT = mybir.dt.bfloat16
F32 = mybir.dt.float32
ActFn = mybir.ActivationFunctionType
NEG = -1e30

FULL = dict(B=2, T=2048, C=4096, H=32, W=256, QT=512)


def _dims(cfg):
    B, T, C, H = cfg["B"], cfg["T"], cfg["C"], cfg["H"]
    W, QT = cfg["W"], cfg["QT"]
    HD = C // H
    assert HD == P
    HL = H // NCORES
    R = B * T
    RS = R // NCORES
    KO = C // P
    assert R % W == 0 and T % QT == 0 and QT % P == 0 and W % P == 0
    assert T % W == 0  # w-blocks may not straddle batches (cos/sin slicing)
    return B, T, C, H, HD, HL, R, RS, KO, W, QT


def build_nc(cfg=FULL):
    B, T, C, H, HD, HL, R, RS, KO, W, QT = _dims(cfg)
    NW = R // W
    NWB = NW // B  # w-blocks per batch
    NKT = T // P
    NQT = T // QT
    QB = QT // P
    HQ = QT // 2  # phase-3 half-tile columns
    NCB2 = C // HQ
    CH = 4  # startup chunk size in ko units
    SCALE = float(HD) ** -0.5

    nc = bacc.Bacc(None, num_devices=NCORES)

    xT = nc.dram_tensor("xT", [P, KO, R], DT, kind="ExternalInput")
    wqT = nc.dram_tensor("wqT", [P, KO, HL * HD], DT, kind="ExternalInput")
    wkT = nc.dram_tensor("wkT", [P, KO, HL * HD], DT, kind="ExternalInput")
    wvT = nc.dram_tensor("wvT", [P, KO, HL * HD], DT, kind="ExternalInput")
    woT = nc.dram_tensor("woT", [P, KO, C], DT, kind="ExternalInput")
    cosT = nc.dram_tensor("cosT", [P, T], DT, kind="ExternalInput")
    sinT = nc.dram_tensor("sinT", [P, T], DT, kind="ExternalInput")
    negT = nc.dram_tensor("negT", [P, P], DT, kind="ExternalInput")
    idT = nc.dram_tensor("idT", [P, P], DT, kind="ExternalInput")
    y = nc.dram_tensor("y", [RS, C], F32, kind="ExternalOutput")

    qT_d = nc.dram_tensor("qT_d", [P, HL, R], DT)
    kT_d = nc.dram_tensor("kT_d", [P, HL, R], DT)
    # two a2a groups: heads {0,1} and heads {2,3}
    a2a_i = [nc.dram_tensor(f"a2a_i{g}", [NCORES, 2 * HD, RS], DT)
             for g in range(2)]
    a2a_o = [nc.dram_tensor(f"a2a_o{g}", [NCORES, 2 * HD, RS], DT)
             for g in range(2)]

    with tile.TileContext(nc) as tc:
        with (
            tc.tile_pool(name="tab", bufs=1) as tab,
            tc.tile_pool(name="attpre", bufs=1) as attpre,
        ):
            ones_sb = tab.tile([P, P], DT, tag="ones")
            nc.vector.memset(ones_sb[:], 1.0)
            neg_sb = tab.tile([P, P], DT, tag="neg")
            nc.gpsimd.dma_start(neg_sb[:], negT[:])
            id_sb = tab.tile([P, P], DT, tag="id")
            nc.gpsimd.dma_start(id_sb[:], idT[:])
            # first-attention tiles in a pool that can't overlap the phase-1
            # pools: their loads run mid-phase-1, killing the transition gap
            kTb0 = attpre.tile([P, T], DT, tag="kTb0")
            qTt0 = attpre.tile([P, QT], DT, tag="qTt0")

            with tc.tile_pool(name="vres", bufs=1) as vres:
                v_sb = vres.tile([P, R // P, HL * HD], DT, tag="v")

                # ---------- phase 1: q/k/v projections + rope ----------
                with (
                    tc.tile_pool(name="cs1", bufs=1) as cs1,
                    tc.tile_pool(name="wp", bufs=1) as wp,
                    tc.tile_pool(name="xp", bufs=2) as xp,
                    tc.tile_pool(name="ev1", bufs=3) as ev1,
                    tc.tile_pool(name="ps1", bufs=1, space="PSUM") as ps1,
                    tc.tile_pool(name="psv", bufs=2, space="PSUM") as psv,
                ):
                    cos_sb = cs1.tile([P, T], DT, tag="cos")
                    sin_sb = cs1.tile([P, T], DT, tag="sin")
                    nc.gpsimd.dma_start(cos_sb[:], cosT[:])
                    nc.gpsimd.dma_start(sin_sb[:], sinT[:])
                    wq_sb = wp.tile([P, KO, HL * HD], DT, tag="wq")
                    wk_sb = wp.tile([P, KO, HL * HD], DT, tag="wk")
                    wv_sb = wp.tile([P, KO, HL * HD], DT, tag="wv")
                    xw0 = xp.tile([P, KO, W], DT, tag="xw")
                    # chunked startup loads on parallel queues
                    for k in range(0, KO, CH):
                        nc.sync.dma_start(
                            wq_sb[:, k:k + CH], wqT[:, k:k + CH])
                    for k in range(0, KO, CH):
                        nc.scalar.dma_start(
                            xw0[:, k:k + CH], xT[:, k:k + CH, 0:W])
                    for k in range(0, KO, CH):
                        nc.scalar.dma_start(
                            wk_sb[:, k:k + CH], wkT[:, k:k + CH])
                    for k in range(0, KO, CH):
                        nc.sync.dma_start(
                            wv_sb[:, k:k + CH], wvT[:, k:k + CH])

                    for w in range(NW):
                        if w == 0:
                            xw = xw0
                        else:
                            xw = xp.tile([P, KO, W], DT, tag="xw")
                            nc.sync.dma_start(
                                xw[:], xT[:, :, w * W:(w + 1) * W])
                        rsl = slice(w * W, (w + 1) * W)
                        t0 = (w * W) % T
                        tsl = slice(t0, t0 + W)

                        for wsb, dst in ((wq_sb, qT_d), (wk_sb, kT_d)):
                            pts = [ps1.tile([P, W], F32, tag=f"p{h}",
                                            name=f"pqk{h}")
                                   for h in range(HL)]
                            for k in range(KO):
                                for h in range(HL):
                                    nc.tensor.matmul(
                                        pts[h][:],
                                        wsb[:, k, h * HD:(h + 1) * HD],
                                        xw[:, k],
                                        start=(k == 0), stop=(k == KO - 1),
                                    )
                            for h in range(HL):
                                raw = ev1.tile([P, W], DT, tag="raw")
                                nc.scalar.activation(
                                    raw[:], pts[h][:], ActFn.Copy)
                                sw = ev1.tile([P, W], DT, tag="sw")
                                nc.sync.dma_start(sw[0:64, :], raw[64:128, :])
                                nc.sync.dma_start(sw[64:128, :], raw[0:64, :])
                                t1 = ev1.tile([P, W], DT, tag="t1")
                                nc.vector.tensor_tensor(
                                    t1[:], sw[:], sin_sb[:, tsl],
                                    mybir.AluOpType.mult)
                                rot = ev1.tile([P, W], DT, tag="rot")
                                nc.vector.tensor_tensor(
                                    rot[:], raw[:], cos_sb[:, tsl],
                                    mybir.AluOpType.mult)
                                nc.vector.tensor_tensor(
                                    rot[:], rot[:], t1[:],
                                    mybir.AluOpType.add)
                                nc.scalar.dma_start(dst[:, h, rsl], rot[:])

                        for rs_ in range(W // P):
                            pt = psv.tile([P, HL * HD], F32, tag="pv")
                            for k in range(KO):
                                nc.tensor.matmul(
                                    pt[:], xw[:, k, rs_ * P:(rs_ + 1) * P],
                                    wv_sb[:, k],
                                    start=(k == 0), stop=(k == KO - 1),
                                )
                            nc.scalar.activation(
                                v_sb[:, w * (W // P) + rs_, :], pt[:],
                                ActFn.Copy)

                        if w == NWB - 1:
                            # batch-0 projections done: prefetch the first
                            # attention operands now
                            nc.sync.dma_start(kTb0[:], kT_d[:, 0, 0:T])
                            nc.sync.dma_start(qTt0[:], qT_d[:, 0, 0:QT])

                # ---------- phase 2: attention ----------
                with (
                    tc.tile_pool(name="att", bufs=3) as att,
                    tc.tile_pool(name="qp", bufs=4) as qp,
                    tc.tile_pool(name="up", bufs=4) as up,
                    tc.tile_pool(name="ps2", bufs=3, space="PSUM") as ps2,
                    tc.tile_pool(name="pso", bufs=3, space="PSUM") as pso,
                    tc.tile_pool(name="psd", bufs=2, space="PSUM") as psd,
                ):
                    for h in range(HL):
                        for b in range(B):
                            if h == 0 and b == 0:
                                kTb = kTb0
                            else:
                                kTb = att.tile([P, T], DT, tag="kTb")
                                nc.sync.dma_start(
                                    kTb[:], kT_d[:, h, b * T:(b + 1) * T])
                            for qt in range(NQT):
                                if h == 0 and b == 0 and qt == 0:
                                    qTt = qTt0
                                else:
                                    qTt = qp.tile([P, QT], DT, tag="qTt")
                                    nc.sync.dma_start(
                                        qTt[:],
                                        qT_d[:, h, b * T + qt * QT:
                                             b * T + (qt + 1) * QT])
                                po = pso.tile([P, QT], F32, tag="po")
                                pd = psd.tile([P, QT], F32, tag="pd")
                                nkt = (qt + 1) * QB

                                def emit_pv_den(pcs, pkt, pu):
                                    first = pkt == 0
                                    last = pkt == nkt - 1
                                    nc.tensor.matmul(
                                        po[:, pcs],
                                        v_sb[:, b * NKT + pkt,
                                             h * HD:(h + 1) * HD],
                                        pu[:, pcs], start=first, stop=last)
                                    nc.tensor.matmul(
                                        pd[:, pcs], ones_sb[:], pu[:, pcs],
                                        start=first, stop=last)

                                prev = None
                                for kt in range(nkt):
                                    j = kt - qt * QB
                                    c0 = P * j if j > 0 else 0
                                    cs = slice(c0, QT)
                                    diag = j >= 0
                                    pS = ps2.tile([P, QT], F32, tag="pS")
                                    nc.tensor.matmul(
                                        pS[:, cs],
                                        kTb[:, kt * P:(kt + 1) * P],
                                        qTt[:, cs],
                                        start=True, stop=not diag,
                                    )
                                    if diag:  # -1e30 upper triangle, on PE
                                        nc.tensor.matmul(
                                            pS[:, c0:c0 + P], neg_sb[:],
                                            id_sb[:],
                                            start=False, stop=True,
                                        )
                                    u = up.tile([P, QT], DT, tag="u")
                                    nc.scalar.activation(
                                        u[:, cs], pS[:, cs], ActFn.Exp,
                                        scale=SCALE)
                                    # PV/denominator lag one block so the exp
                                    # hides behind the next score matmul
                                    if prev is not None:
                                        emit_pv_den(*prev)
                                    prev = (cs, kt, u)
                                emit_pv_den(*prev)
                                rec = att.tile([P, QT], F32, tag="rec")
                                nc.vector.reciprocal_approx_fast(
                                    rec[:], pd[:])
                                ot = att.tile([P, QT], DT, tag="ot")
                                nc.vector.tensor_tensor(
                                    ot[:], po[:], rec[:],
                                    mybir.AluOpType.mult)
                                g, hh = divmod(h, 2)
                                nc.sync.dma_start(
                                    a2a_i[g][b * NQT + qt,
                                             hh * HD:(hh + 1) * HD, :],
                                    ot[:])
                        if h == 1 or h == 3:
                            g = h // 2
                            nc.gpsimd.collective_compute(
                                "AllToAll",
                                mybir.AluOpType.bypass,
                                replica_groups=[list(range(NCORES))],
                                ins=[a2a_i[g][:]],
                                outs=[a2a_o[g][:]],
                            )

            # ---------- phase 3: output projection, two passes ----------
            with (
                tc.tile_pool(name="a3", bufs=1) as a3,
                tc.tile_pool(name="wop", bufs=4) as wop,
                tc.tile_pool(name="yacc", bufs=1) as yacc,
                tc.tile_pool(name="yp", bufs=3) as yp,
                tc.tile_pool(name="ps3", bufs=3, space="PSUM") as ps3,
            ):
                # aT layout: [dim, core, local-head-in-group, row]
                aT01 = a3.tile([P, NCORES, 2, RS], DT, tag="aT01")
                nc.gpsimd.dma_start(
                    aT01[:], a2a_o[0][:].rearrange("s (i d) r -> d s i r",
                                                   d=P))
                aT23 = a3.tile([P, NCORES, 2, RS], DT, tag="aT23")
                nc.gpsimd.dma_start(
                    aT23[:], a2a_o[1][:].rearrange("s (i d) r -> d s i r",
                                                   d=P))
                ya_tiles = []
                # pass A: heads {0,1} -> bf16 partials in SBUF (runs while
                # a2a of heads {2,3} is in flight)
                for cb in range(NCB2):
                    wot = wop.tile([P, KO, HQ], DT, tag="wot", name="wota")
                    nc.sync.dma_start(
                        wot[:], woT[:, :, cb * HQ:(cb + 1) * HQ])
                    for rs_ in range(RS // P):
                        pt = ps3.tile([P, HQ], F32, tag="py")
                        n = 0
                        for s in range(NCORES):
                            for i in range(2):
                                nc.tensor.matmul(
                                    pt[:],
                                    aT01[:, s, i, rs_ * P:(rs_ + 1) * P],
                                    wot[:, s * HL + i],
                                    start=(n == 0),
                                    stop=(n == 2 * NCORES - 1),
                                )
                                n += 1
                        ya = yacc.tile([P, HQ], DT, tag=f"ya{cb}_{rs_}",
                                       name=f"ya{cb}_{rs_}")
                        nc.scalar.activation(ya[:], pt[:], ActFn.Copy)
                        ya_tiles.append(ya)
                # pass B: heads {2,3} + merge + store
                for cb in range(NCB2):
                    wot = wop.tile([P, KO, HQ], DT, tag="wot", name="wotb")
                    nc.sync.dma_start(
                        wot[:], woT[:, :, cb * HQ:(cb + 1) * HQ])
                    for rs_ in range(RS // P):
                        pt = ps3.tile([P, HQ], F32, tag="py")
                        n = 0
                        for s in range(NCORES):
                            for i in range(2):
                                nc.tensor.matmul(
                                    pt[:],
                                    aT23[:, s, i, rs_ * P:(rs_ + 1) * P],
                                    wot[:, s * HL + 2 + i],
                                    start=(n == 0),
                                    stop=(n == 2 * NCORES - 1),
                                )
                                n += 1
                        yt = yp.tile([P, HQ], F32, tag="yt")
                        nc.vector.tensor_tensor(
                            yt[:], pt[:],
                            ya_tiles[cb * (RS // P) + rs_][:],
                            mybir.AluOpType.add)
                        nc.scalar.dma_start(
                            y[rs_ * P:(rs_ + 1) * P,
                              cb * HQ:(cb + 1) * HQ], yt[:])

    nc.compile()
    return nc


def _as_lhsT_tiles(w):
    """[M, K] row-major -> [P, K//P, M]: out[p, ko, m] = w[m, ko*P + p]."""
    M, K = w.shape
    return np.ascontiguousarray(
        w.reshape(M, K // P, P).transpose(2, 1, 0)).astype(BF16)


def prep_inputs(x, wq, wk, wv, wo, cfg=FULL):
    B, T, C, H, HD, HL, R, RS, KO, W, QT = _dims(cfg)
    rope_perm = np.concatenate([np.arange(0, HD, 2), np.arange(1, HD, 2)])

    xflat = np.ascontiguousarray(x.reshape(R, C))
    xT = _as_lhsT_tiles(xflat)                       # [P, KO, R]
    woT = _as_lhsT_tiles(wo)                         # [P, KO, C]

    t = np.arange(T, dtype=np.float64)
    cosT = np.broadcast_to(np.cos(t), (P, T)).astype(BF16)
    sin_row = np.sin(t)
    sinT = np.empty((P, T), np.float64)
    sinT[0:64, :] = -sin_row
    sinT[64:128, :] = sin_row
    sinT = sinT.astype(BF16)

    # mask matmul constants: out[m, c] = negT[c, m] must be NEG iff c < m
    cc = np.arange(P)
    negT = np.where(cc[:, None] < cc[None, :], NEG, 0.0).astype(BF16)
    idT = np.eye(P).astype(BF16)

    per_core = []
    for m in range(NCORES):
        sl = slice(m * HL * HD, (m + 1) * HL * HD)
        wq_m = wq[sl].reshape(HL, HD, C)[:, rope_perm, :].reshape(HL * HD, C)
        wk_m = wk[sl].reshape(HL, HD, C)[:, rope_perm, :].reshape(HL * HD, C)
        per_core.append(dict(
            xT=xT,
            wqT=_as_lhsT_tiles(wq_m),
            wkT=_as_lhsT_tiles(wk_m),
            wvT=_as_lhsT_tiles(wv[sl]),
            woT=woT,
            cosT=cosT,
            sinT=sinT,
            negT=negT,
            idT=idT,
        ))
    return per_core


_NC_CACHE = None
LAST_EXEC_NS = None
LAST_RESULT = None


def kernel(x, wq, wk, wv, wo):
    global _NC_CACHE, LAST_EXEC_NS, LAST_RESULT
    cfg = FULL
    B, T, C = cfg["B"], cfg["T"], cfg["C"]
    if _NC_CACHE is None:
        _NC_CACHE = build_nc(cfg)
    nc = _NC_CACHE
    in_maps = prep_inputs(
        np.asarray(x, np.float32), np.asarray(wq, np.float32),
        np.asarray(wk, np.float32), np.asarray(wv, np.float32),
        np.asarray(wo, np.float32), cfg)
    res = run_bass_kernel_spmd(nc, in_maps, core_ids=list(range(NCORES)))
    LAST_RESULT = res
    if res.exec_time_ns is not None:
        LAST_EXEC_NS = res.exec_time_ns
    y = np.concatenate([r["y"] for r in res.results], axis=0)
    return y.reshape(B, T, C).astype(np.float32)


# revision 24
# speedup vs baseline: 1.1507x; 1.0621x over previous
"""Multi-head causal attention (B=2, T=2048, C=4096, H=32) on 8 Trainium2
NeuronCores, tensor-parallel over heads (Megatron-style).

Per core m (4 heads each):
  phase 1: q/k/v projections from full x (weights column-sharded,
           host-pre-transposed into lhsT/rhs layouts). k-outer MM ordering
           streams against chunked (4-ko) weight/x DMAs on parallel queues so
           the PE starts ~1us in. RoPE applied at PSUM eviction (all rotary
           freqs == 1.0, so cos/sin are per-position scalars; head_dim
           host-permuted to [evens, odds]; the half-swap runs through
           SBUF->SBUF DMA). v is evicted directly into a persistent SBUF
           tile (no DRAM trip).
  phase 2: attention per (head, batch) with scores TRANSPOSED [k, q]:
           u = exp(scale * sT) (no max subtraction needed at these scales).
           Blocks above the causal diagonal are skipped; diagonal blocks are
           column-trimmed. The causal mask is applied ON the tensor engine: a
           constant [-1e30 upper-triangle] stationary matrix against identity
           accumulates into the diagonal 128-block of the scores PSUM
           (reference's additive NEG_INF semantics). The PV/denominator
           matmuls are emitted one block behind the scores matmul so the exp
           latency hides behind the next score block. Softmax denominator
           accumulates in PSUM via trimmed all-ones matmuls; normalization
           uses the single-op approx reciprocal.
  a2a:     TWO AllToAlls (heads {0,1} fire mid-phase-2, heads {2,3} at the
           end) - per-collective cost here is ~fixed, so the first hides
           under attention of heads 2/3 and the second under the first half
           of phase 3.
  phase 3: y_rows = a_rows @ wo.T in two half-contraction passes: pass A
           accumulates heads {0,1} into bf16 SBUF partials while a2a(23) is
           in flight; pass B adds heads {2,3} and stores. wo streams twice
           as 16 half-tiles per pass with 4 buffers so loads stay ahead of
           consumption.
Host gathers the 8 row-slices; host does layout prep and the final concat.
"""

import os
import sys

import numpy as np

for _p in ("/opt/trn_rl_repo", "/root/.axon_site/_ro/trn_rl_repo"):
    if os.path.isdir(_p) and _p not in sys.path:
        sys.path.insert(0, _p)

import ml_dtypes

import concourse.bacc as bacc
import concourse.bass as bass
import concourse.mybir as mybir
import concourse.tile as tile
from concourse.bass_utils import run_bass_kernel_spmd

BF16 = ml_dtypes.bfloat16
P = 128
NCORES = 8
DT = mybir.dt.bfloat16
F32 = mybir.dt.float32
ActFn = mybir.ActivationFunctionType
NEG = -1e30

FULL = dict(B=2, T=2048, C=4096, H=32, W=256, QT=512)


def _dims(cfg):
    B, T, C, H = cfg["B"], cfg["T"], cfg["C"], cfg["H"]
    W, QT = cfg["W"], cfg["QT"]
    HD = C // H
    assert HD == P
    HL = H // NCORES
    R = B * T
    RS = R // NCORES
    KO = C // P
    assert R % W == 0 and T % QT == 0 and QT % P == 0 and W % P == 0
    assert T % W == 0  # w-blocks may not straddle batches (cos/sin slicing)
    return B, T, C, H, HD, HL, R, RS, KO, W, QT


def build_nc(cfg=FULL):
    B, T, C, H, HD, HL, R, RS, KO, W, QT = _dims(cfg)
    NW = R // W
    NWB = NW // B  # w-blocks per batch
    NKT = T // P
    NQT = T // QT
    QB = QT // P
    HQ = QT // 2  # phase-3 half-tile columns
    NCB2 = C // HQ
    CH = 4  # startup chunk size in ko units
    SCALE = float(HD) ** -0.5

    nc = bacc.Bacc(None, num_devices=NCORES)

    # x and wo are pre-tiled host-side so every device load is contiguous
    xT = nc.dram_tensor("xT", [P, NW, KO, W], DT, kind="ExternalInput")
    wqT = nc.dram_tensor("wqT", [P, KO, HL * HD], DT, kind="ExternalInput")
    wkT = nc.dram_tensor("wkT", [P, KO, HL * HD], DT, kind="ExternalInput")
    wvT = nc.dram_tensor("wvT", [P, KO, HL * HD], DT, kind="ExternalInput")
    woT = nc.dram_tensor("woT", [P, NCB2, KO, HQ], DT, kind="ExternalInput")
    cosT = nc.dram_tensor("cosT", [P, T], DT, kind="ExternalInput")
    sinT = nc.dram_tensor("sinT", [P, T], DT, kind="ExternalInput")
    negT = nc.dram_tensor("negT", [P, P], DT, kind="ExternalInput")
    idT = nc.dram_tensor("idT", [P, P], DT, kind="ExternalInput")
    y = nc.dram_tensor("y", [RS, C], F32, kind="ExternalOutput")

    qT_d = nc.dram_tensor("qT_d", [P, HL, R], DT)
    kT_d = nc.dram_tensor("kT_d", [P, HL, R], DT)
    # two a2a groups: heads {0,1} and heads {2,3}
    a2a_i = [nc.dram_tensor(f"a2a_i{g}", [NCORES, 2 * HD, RS], DT)
             for g in range(2)]
    a2a_o = [nc.dram_tensor(f"a2a_o{g}", [NCORES, 2 * HD, RS], DT)
             for g in range(2)]

    with tile.TileContext(nc) as tc:
        with (
            tc.tile_pool(name="tab", bufs=1) as tab,
            tc.tile_pool(name="attpre", bufs=1) as attpre,
        ):
            ones_sb = tab.tile([P, P], DT, tag="ones")
            nc.vector.memset(ones_sb[:], 1.0)
            neg_sb = tab.tile([P, P], DT, tag="neg")
            nc.gpsimd.dma_start(neg_sb[:], negT[:])
            id_sb = tab.tile([P, P], DT, tag="id")
            nc.gpsimd.dma_start(id_sb[:], idT[:])
            # first-attention tiles in a pool that can't overlap the phase-1
            # pools: their loads run mid-phase-1, killing the transition gap
            kTb0 = attpre.tile([P, T], DT, tag="kTb0")
            qTt0 = attpre.tile([P, QT], DT, tag="qTt0")

            if True:
                # manual lifetimes (stack order!): a3 outlives vres, so it is
                # allocated first; vres releases before phase 3 so its space
                # feeds yacc. One shared aT tile: heads {0,1} during pass A,
                # reloaded (WAR-ordered) with heads {2,3} for pass B.
                a3 = tc.alloc_tile_pool(name="a3", bufs=1)
                aT = a3.tile([P, NCORES, 2, RS], DT, tag="aT")
                vres = tc.alloc_tile_pool(name="vres", bufs=1)
                v_sb = vres.tile([P, R // P, HL * HD], DT, tag="v")

                # ---------- phase 1: q/k/v projections + rope ----------
                with (
                    tc.tile_pool(name="cs1", bufs=1) as cs1,
                    tc.tile_pool(name="wp", bufs=1) as wp,
                    tc.tile_pool(name="xp", bufs=2) as xp,
                    tc.tile_pool(name="ev1", bufs=3) as ev1,
                    tc.tile_pool(name="ps1", bufs=1, space="PSUM") as ps1,
                    tc.tile_pool(name="psv", bufs=2, space="PSUM") as psv,
                ):
                    cos_sb = cs1.tile([P, T], DT, tag="cos")
                    sin_sb = cs1.tile([P, T], DT, tag="sin")
                    nc.gpsimd.dma_start(cos_sb[:], cosT[:])
                    nc.gpsimd.dma_start(sin_sb[:], sinT[:])
                    wq_sb = wp.tile([P, KO, HL * HD], DT, tag="wq")
                    wk_sb = wp.tile([P, KO, HL * HD], DT, tag="wk")
                    wv_sb = wp.tile([P, KO, HL * HD], DT, tag="wv")
                    xw0 = xp.tile([P, KO, W], DT, tag="xw")
                    # chunked startup loads on parallel queues
                    for k in range(0, KO, CH):
                        nc.sync.dma_start(
                            wq_sb[:, k:k + CH], wqT[:, k:k + CH])
                    for k in range(0, KO, CH):
                        nc.scalar.dma_start(
                            xw0[:, k:k + CH], xT[:, 0, k:k + CH])
                    for k in range(0, KO, CH):
                        nc.scalar.dma_start(
                            wk_sb[:, k:k + CH], wkT[:, k:k + CH])
                    for k in range(0, KO, CH):
                        nc.sync.dma_start(
                            wv_sb[:, k:k + CH], wvT[:, k:k + CH])

                    for w in range(NW):
                        if w == 0:
                            xw = xw0
                        else:
                            xw = xp.tile([P, KO, W], DT, tag="xw")
                            nc.sync.dma_start(xw[:], xT[:, w])
                        rsl = slice(w * W, (w + 1) * W)
                        t0 = (w * W) % T
                        tsl = slice(t0, t0 + W)

                        for wsb, dst in ((wq_sb, qT_d), (wk_sb, kT_d)):
                            pts = [ps1.tile([P, W], F32, tag=f"p{h}",
                                            name=f"pqk{h}")
                                   for h in range(HL)]
                            for k in range(KO):
                                for h in range(HL):
                                    nc.tensor.matmul(
                                        pts[h][:],
                                        wsb[:, k, h * HD:(h + 1) * HD],
                                        xw[:, k],
                                        start=(k == 0), stop=(k == KO - 1),
                                    )
                            for h in range(HL):
                                raw = ev1.tile([P, W], DT, tag="raw")
                                nc.scalar.activation(
                                    raw[:], pts[h][:], ActFn.Copy)
                                sw = ev1.tile([P, W], DT, tag="sw")
                                nc.sync.dma_start(sw[0:64, :], raw[64:128, :])
                                nc.sync.dma_start(sw[64:128, :], raw[0:64, :])
                                t1 = ev1.tile([P, W], DT, tag="t1")
                                nc.vector.tensor_tensor(
                                    t1[:], sw[:], sin_sb[:, tsl],
                                    mybir.AluOpType.mult)
                                rot = ev1.tile([P, W], DT, tag="rot")
                                nc.vector.tensor_tensor(
                                    rot[:], raw[:], cos_sb[:, tsl],
                                    mybir.AluOpType.mult)
                                nc.vector.tensor_tensor(
                                    rot[:], rot[:], t1[:],
                                    mybir.AluOpType.add)
                                nc.scalar.dma_start(dst[:, h, rsl], rot[:])

                        for rs_ in range(W // P):
                            pt = psv.tile([P, HL * HD], F32, tag="pv")
                            for k in range(KO):
                                nc.tensor.matmul(
                                    pt[:], xw[:, k, rs_ * P:(rs_ + 1) * P],
                                    wv_sb[:, k],
                                    start=(k == 0), stop=(k == KO - 1),
                                )
                            nc.scalar.activation(
                                v_sb[:, w * (W // P) + rs_, :], pt[:],
                                ActFn.Copy)

                        if w == NWB - 1:
                            # batch-0 projections done: prefetch the first
                            # attention operands now
                            nc.sync.dma_start(kTb0[:], kT_d[:, 0, 0:T])
                            nc.sync.dma_start(qTt0[:], qT_d[:, 0, 0:QT])

                # ---------- phase 2: attention ----------
                with (
                    tc.tile_pool(name="att", bufs=3) as att,
                    tc.tile_pool(name="qp", bufs=4) as qp,
                    tc.tile_pool(name="up", bufs=4) as up,
                    tc.tile_pool(name="ps2", bufs=3, space="PSUM") as ps2,
                    tc.tile_pool(name="pso", bufs=3, space="PSUM") as pso,
                    tc.tile_pool(name="psd", bufs=2, space="PSUM") as psd,
                ):
                    for h in range(HL):
                        for b in range(B):
                            if h == 0 and b == 0:
                                kTb = kTb0
                            else:
                                kTb = att.tile([P, T], DT, tag="kTb")
                                nc.sync.dma_start(
                                    kTb[:], kT_d[:, h, b * T:(b + 1) * T])
                            for qt in range(NQT):
                                if h == 0 and b == 0 and qt == 0:
                                    qTt = qTt0
                                else:
                                    qTt = qp.tile([P, QT], DT, tag="qTt")
                                    nc.sync.dma_start(
                                        qTt[:],
                                        qT_d[:, h, b * T + qt * QT:
                                             b * T + (qt + 1) * QT])
                                po = pso.tile([P, QT], F32, tag="po")
                                pd = psd.tile([P, QT], F32, tag="pd")
                                nkt = (qt + 1) * QB

                                def emit_pv_den(pcs, pkt, pu):
                                    first = pkt == 0
                                    last = pkt == nkt - 1
                                    nc.tensor.matmul(
                                        po[:, pcs],
                                        v_sb[:, b * NKT + pkt,
                                             h * HD:(h + 1) * HD],
                                        pu[:, pcs], start=first, stop=last)
                                    nc.tensor.matmul(
                                        pd[:, pcs], ones_sb[:], pu[:, pcs],
                                        start=first, stop=last)

                                prev = None
                                for kt in range(nkt):
                                    j = kt - qt * QB
                                    c0 = P * j if j > 0 else 0
                                    cs = slice(c0, QT)
                                    diag = j >= 0
                                    pS = ps2.tile([P, QT], F32, tag="pS")
                                    nc.tensor.matmul(
                                        pS[:, cs],
                                        kTb[:, kt * P:(kt + 1) * P],
                                        qTt[:, cs],
                                        start=True, stop=not diag,
                                    )
                                    if diag:  # -1e30 upper triangle, on PE
                                        nc.tensor.matmul(
                                            pS[:, c0:c0 + P], neg_sb[:],
                                            id_sb[:],
                                            start=False, stop=True,
                                        )
                                    u = up.tile([P, QT], DT, tag="u")
                                    nc.scalar.activation(
                                        u[:, cs], pS[:, cs], ActFn.Exp,
                                        scale=SCALE)
                                    # PV/denominator lag one block so the exp
                                    # hides behind the next score matmul
                                    if prev is not None:
                                        emit_pv_den(*prev)
                                    prev = (cs, kt, u)
                                emit_pv_den(*prev)
                                rec = att.tile([P, QT], F32, tag="rec")
                                nc.vector.reciprocal_approx_fast(
                                    rec[:], pd[:])
                                ot = att.tile([P, QT], DT, tag="ot")
                                nc.vector.tensor_tensor(
                                    ot[:], po[:], rec[:],
                                    mybir.AluOpType.mult)
                                g, hh = divmod(h, 2)
                                nc.sync.dma_start(
                                    a2a_i[g][b * NQT + qt,
                                             hh * HD:(hh + 1) * HD, :],
                                    ot[:])
                        if h == 1 or h == 3:
                            g = h // 2
                            nc.gpsimd.collective_compute(
                                "AllToAll",
                                mybir.AluOpType.bypass,
                                replica_groups=[list(range(NCORES))],
                                ins=[a2a_i[g][:]],
                                outs=[a2a_o[g][:]],
                            )
                            if g == 0:
                                nc.gpsimd.dma_start(
                                    aT[:],
                                    a2a_o[0][:].rearrange(
                                        "s (i d) r -> d s i r", d=P))

            vres.release()

            # ---------- phase 3: output projection, two passes ----------
            with (
                tc.tile_pool(name="wop", bufs=4) as wop,
                tc.tile_pool(name="yacc", bufs=1) as yacc,
                tc.tile_pool(name="yp", bufs=3) as yp,
                tc.tile_pool(name="ps3", bufs=3, space="PSUM") as ps3,
            ):
                ya_tiles = []
                # pass A: heads {0,1} -> bf16 partials in SBUF (runs while
                # a2a of heads {2,3} is in flight)
                for cb in range(NCB2):
                    wot = wop.tile([P, KO, HQ], DT, tag="wot", name="wota")
                    nc.sync.dma_start(
                        wot[:], woT[:, cb])
                    for rs_ in range(RS // P):
                        pt = ps3.tile([P, HQ], F32, tag="py")
                        n = 0
                        for s in range(NCORES):
                            for i in range(2):
                                nc.tensor.matmul(
                                    pt[:],
                                    aT[:, s, i, rs_ * P:(rs_ + 1) * P],
                                    wot[:, s * HL + i],
                                    start=(n == 0),
                                    stop=(n == 2 * NCORES - 1),
                                )
                                n += 1
                        ya = yacc.tile([P, HQ], DT, tag=f"ya{cb}_{rs_}",
                                       name=f"ya{cb}_{rs_}")
                        nc.scalar.activation(ya[:], pt[:], ActFn.Copy)
                        ya_tiles.append(ya)
                # pass B: heads {2,3} + merge + store
                nc.gpsimd.dma_start(
                    aT[:],
                    a2a_o[1][:].rearrange("s (i d) r -> d s i r", d=P))
                for cb in range(NCB2):
                    wot = wop.tile([P, KO, HQ], DT, tag="wot", name="wotb")
                    nc.sync.dma_start(
                        wot[:], woT[:, cb])
                    for rs_ in range(RS // P):
                        pt = ps3.tile([P, HQ], F32, tag="py")
                        n = 0
                        for s in range(NCORES):
                            for i in range(2):
                                nc.tensor.matmul(
                                    pt[:],
                                    aT[:, s, i, rs_ * P:(rs_ + 1) * P],
                                    wot[:, s * HL + 2 + i],
                                    start=(n == 0),
                                    stop=(n == 2 * NCORES - 1),
                                )
                                n += 1
                        yt = yp.tile([P, HQ], F32, tag="yt")
                        nc.vector.tensor_tensor(
                            yt[:], pt[:],
                            ya_tiles[cb * (RS // P) + rs_][:],
                            mybir.AluOpType.add)
                        nc.scalar.dma_start(
                            y[rs_ * P:(rs_ + 1) * P,
                              cb * HQ:(cb + 1) * HQ], yt[:])

            a3.release()

    nc.compile()
    return nc


def _as_lhsT_tiles(w):
    """[M, K] row-major -> [P, K//P, M]: out[p, ko, m] = w[m, ko*P + p]."""
    M, K = w.shape
    return np.ascontiguousarray(
        w.reshape(M, K // P, P).transpose(2, 1, 0)).astype(BF16)


def prep_inputs(x, wq, wk, wv, wo, cfg=FULL):
    B, T, C, H, HD, HL, R, RS, KO, W, QT = _dims(cfg)
    rope_perm = np.concatenate([np.arange(0, HD, 2), np.arange(1, HD, 2)])

    NW = R // W
    HQ = QT // 2
    NCB2 = C // HQ
    xflat = np.ascontiguousarray(x.reshape(R, C))
    # [P, KO, R] -> [P, NW, KO, W] so each w-block load is contiguous
    xT = np.ascontiguousarray(
        _as_lhsT_tiles(xflat).reshape(P, KO, NW, W).transpose(0, 2, 1, 3))
    # [P, KO, C] -> [P, NCB2, KO, HQ] so each wo half-tile is contiguous
    woT = np.ascontiguousarray(
        _as_lhsT_tiles(wo).reshape(P, KO, NCB2, HQ).transpose(0, 2, 1, 3))

    t = np.arange(T, dtype=np.float64)
    cosT = np.broadcast_to(np.cos(t), (P, T)).astype(BF16)
    sin_row = np.sin(t)
    sinT = np.empty((P, T), np.float64)
    sinT[0:64, :] = -sin_row
    sinT[64:128, :] = sin_row
    sinT = sinT.astype(BF16)

    # mask matmul constants: out[m, c] = negT[c, m] must be NEG iff c < m
    cc = np.arange(P)
    negT = np.where(cc[:, None] < cc[None, :], NEG, 0.0).astype(BF16)
    idT = np.eye(P).astype(BF16)

    per_core = []
    for m in range(NCORES):
        sl = slice(m * HL * HD, (m + 1) * HL * HD)
        wq_m = wq[sl].reshape(HL, HD, C)[:, rope_perm, :].reshape(HL * HD, C)
        wk_m = wk[sl].reshape(HL, HD, C)[:, rope_perm, :].reshape(HL * HD, C)
        per_core.append(dict(
            xT=xT,
            wqT=_as_lhsT_tiles(wq_m),
            wkT=_as_lhsT_tiles(wk_m),
            wvT=_as_lhsT_tiles(wv[sl]),
            woT=woT,
            cosT=cosT,
            sinT=sinT,
            negT=negT,
            idT=idT,
        ))
    return per_core


_NC_CACHE = None
LAST_EXEC_NS = None
LAST_RESULT = None


def kernel(x, wq, wk, wv, wo):
    global _NC_CACHE, LAST_EXEC_NS, LAST_RESULT
    cfg = FULL
    B, T, C = cfg["B"], cfg["T"], cfg["C"]
    if _NC_CACHE is None:
        _NC_CACHE = build_nc(cfg)
    nc = _NC_CACHE
    in_maps = prep_inputs(
        np.asarray(x, np.float32), np.asarray(wq, np.float32),
        np.asarray(wk, np.float32), np.asarray(wv, np.float32),
        np.asarray(wo, np.float32), cfg)
    res = run_bass_kernel_spmd(nc, in_maps, core_ids=list(range(NCORES)))
    LAST_RESULT = res
    if res.exec_time_ns is not None:
        LAST_EXEC_NS = res.exec_time_ns
    y = np.concatenate([r["y"] for r in res.results], axis=0)
    return y.reshape(B, T, C).astype(np.float32)
